# revision 2
# baseline (speedup 1.0000x reference)
"""Self-contained MHA kernel for Trainium2, 8 NeuronCores — v3.

Problem: B=4, T=2048, D=1024, H=16 causal MHA, fp32, no bias.
Sharding: core c handles batch b=c//2 and head-group hg=c%2 (8 heads = 4
head-pairs), Megatron-style: Wq/Wk/Wv column-sharded, Wo row-sharded; host
sums the two partial outputs per batch (and divides by the 32x weight
pre-scale).

Design notes:
 - projections and out-projection run as fp8e4m3 hi/lo 3-term DoubleRow
   matmuls (2 k-tiles per instruction at 0.5 cyc/row): x = xh+xl, W' = 32W
   = wh+wl (the 32x pre-scale keeps the lo residuals out of fp8 subnormals;
   the scale is undone via the exp scale, the ones column, and a host-side
   divide). Terms: xh*wh + xl*wh + xh*wl.
 - scores S^T[k,q] in bf16 (2 heads via PE quadrants), exp on ACT with
   scale 2^-13 (= 1/8 softmax scale / 32^2), no max subtraction
 - ctx: se tile [k,q] is the STATIONARY, [V|32] the moving operand ->
   ctx[q, dv+1] accumulated over k-tiles, 65 moving rows per (ktile, qtile,
   head), fused denominator column; groups run strictly sequentially per
   PSUM bank (start=True marks the whole 2KB bank pending-zero)
 - V projected directly into [t, dv] layout (x-tile stationary)
 - ctx normalized on DVE (reciprocal + broadcast multiply), transposed
   [q,dv]->[dv,q] by the DMA xbar, then split to fp8 hi/lo for the
   out-projection
"""

import os

import numpy as np

B, T, D, H = 4, 2048, 1024, 16
SCHED_EVERY = int(os.environ.get("K_EVERY", "2"))
SCHED_EARLY = int(os.environ.get("K_EARLY", "1"))
SCHED_CTXI = int(os.environ.get("K_CTXI", "0"))
SCHED_JPOP = int(os.environ.get("K_JPOP", "1"))
SCHED_P3 = int(os.environ.get("K_P3", "0"))
SCHED_WARM = int(os.environ.get("K_WARM", "40"))
SCHED_ROWS = os.environ.get("K_ROWS", "0123")
DK = 64
NCORES = 8
NPAIR = 4        # head-pairs per core
ESH = 512        # output-feature shard per core (8 heads * 64)
VW = 65          # dv + 1 ones column (denominator)
WSCALE = 32.0    # weight pre-scale (keeps fp8 lo parts out of subnormals)

_nc_cache = None


def _build():
    global _nc_cache
    if _nc_cache is not None:
        return _nc_cache

    from contextlib import ExitStack

    import concourse.bacc as bacc
    import concourse.mybir as mybir
    import concourse.tile as tile
    from concourse.masks import make_upper_triangular

    F32 = mybir.dt.float32
    BF16 = mybir.dt.bfloat16
    FP8 = mybir.dt.float8e4
    AF = mybir.ActivationFunctionType
    OP = mybir.AluOpType
    DR = mybir.MatmulPerfMode.DoubleRow

    nc = bacc.Bacc("TRN2", target_bir_lowering=False, debug=False,
                   num_devices=NCORES)
    xh_d = nc.declare_dram_parameter("xh", [D, T], FP8, isOutput=False)
    xl_d = nc.declare_dram_parameter("xl", [D, T], FP8, isOutput=False)
    w_ds = {}
    for wn in ("qh", "ql", "kh", "kl", "vh", "vl"):
        w_ds[wn] = nc.declare_dram_parameter(
            f"w{wn}", [NPAIR, 128, 8, 128], FP8, isOutput=False)
    woh_d = nc.declare_dram_parameter("woh", [128, NPAIR, D], FP8,
                                      isOutput=False)
    wol_d = nc.declare_dram_parameter("wol", [128, NPAIR, D], FP8,
                                      isOutput=False)
    out_d = nc.declare_dram_parameter("out", [T, D], BF16, isOutput=True)

    with tile.TileContext(nc) as tc, ExitStack() as ctx:
        const_p = ctx.enter_context(tc.tile_pool(name="const", bufs=1))
        xt_p = ctx.enter_context(tc.tile_pool(name="xt", bufs=2))
        w_p = ctx.enter_context(tc.tile_pool(name="w", bufs=24))
        wo_p = ctx.enter_context(tc.tile_pool(name="wo", bufs=2))
        qk_p = ctx.enter_context(tc.tile_pool(name="qk", bufs=6))
        v_p = ctx.enter_context(tc.tile_pool(name="v", bufs=4))
        se_p = ctx.enter_context(tc.tile_pool(name="se", bufs=28))
        cu_p = ctx.enter_context(tc.tile_pool(name="cu", bufs=2))
        cb_p = ctx.enter_context(tc.tile_pool(name="cb", bufs=2))
        rec_p = ctx.enter_context(tc.tile_pool(name="rec", bufs=2))
        ctxT_p = ctx.enter_context(tc.tile_pool(name="ctxT", bufs=1))
        st_p = ctx.enter_context(tc.tile_pool(name="st", bufs=3))
        psum_pr = ctx.enter_context(tc.tile_pool(name="psum_pr", bufs=2,
                                                 space="PSUM"))
        psum_ss = ctx.enter_context(tc.tile_pool(name="psum_ss", bufs=2,
                                                 space="PSUM"))
        psum_cx = ctx.enter_context(tc.tile_pool(name="psum_cx", bufs=1,
                                                 space="PSUM"))

        # constants
        trimask = const_p.tile([128, 128], BF16)
        make_upper_triangular(nc, trimask[:], val=1.0, diag=True)

        # persistent tensors
        xh = xt_p.tile([128, 8, T], FP8, tag="xt", name="xh")
        xl = xt_p.tile([128, 8, T], FP8, tag="xt", name="xl")
        ctxTh = ctxT_p.tile([128, NPAIR, T], FP8, tag="ctxTh", name="ctxTh")
        ctxTl = ctxT_p.tile([128, NPAIR, T], FP8, tag="ctxTl", name="ctxTl")

        W_TILES = {}   # p -> dict of 6 hi/lo weight tiles
        QKV = {}       # p -> (QT, KT, V)
        WO = {}
        PROJ_PS = {}

        def emit_x_dma(c):
            csl = slice(c * 512, (c + 1) * 512)
            for t_sb, t_d in ((xh, xh_d), (xl, xl_d)):
                nc.sync.dma_start(
                    out=t_sb[:, :, csl],
                    in_=t_d[:, csl].rearrange("(a p) t -> p a t", p=128))

        def emit_w_dma(p):
            tiles = {}
            for wn in ("qh", "ql", "kh", "kl", "vh", "vl"):
                w = w_p.tile([128, 8, 128], FP8, tag="w", name=f"w{wn}{p}")
                nc.sync.dma_start(out=w[:], in_=w_ds[wn][p])
                tiles[wn] = w
            W_TILES[p] = tiles

        def emit_wo_dma():
            woh = wo_p.tile([128, NPAIR, D], FP8, tag="wo", name="woh")
            wol = wo_p.tile([128, NPAIR, D], FP8, tag="wo", name="wol")
            nc.sync.dma_start(out=woh[:], in_=woh_d[:, :, :])
            nc.sync.dma_start(out=wol[:], in_=wol_d[:, :, :])
            WO["h"] = woh
            WO["l"] = wol

        def alloc_qkv(p):
            QT = qk_p.tile([128, T], BF16, tag="qk", name=f"QT{p}")
            KT = qk_p.tile([128, T], BF16, tag="qk", name=f"KT{p}")
            V = v_p.tile([128, 16, 2, VW], BF16, tag="v", name=f"V{p}")
            nc.gpsimd.memset(V[:, :, :, 64:65], WSCALE)
            QKV[p] = (QT, KT, V)

        # hi/lo 3-term schedule: (x_hi, w_hi), (x_lo, w_hi), (x_hi, w_lo)
        TERMS = (("h", "h"), ("l", "h"), ("h", "l"))

        def _qkproj_unit(p, which, c, half):
            """half of a 512-col chunk of the Q or K projection (fp8 DR)."""
            tiles = W_TILES[p]
            QT, KT, V = QKV[p]
            dst = QT if which == "q" else KT
            csl = slice(c * 512, (c + 1) * 512)
            xts = {"h": xh, "l": xl}
            if half == 0:
                ps = psum_pr.tile([128, 512], F32, tag="pr",
                                  name=f"ps{p}{which}{c}")
                PROJ_PS[(p, which, c)] = ps
                terms = TERMS[:2][:1]  # first term only: 4 DR matmuls
                first = True
            else:
                ps = PROJ_PS.pop((p, which, c))
                terms = TERMS[1:]      # remaining two terms: 8 DR matmuls
                first = False
            for ti, (xs, ws) in enumerate(terms):
                w_sb = tiles[which + ws]
                x_sb = xts[xs]
                last_term = (half == 1 and ti == len(terms) - 1)
                for a in range(0, 8, 2):
                    nc.tensor.matmul(ps[:], w_sb[:, a:a + 2, :],
                                     x_sb[:, a:a + 2, csl],
                                     start=(first and a == 0),
                                     stop=(last_term and a == 6),
                                     perf_mode=DR)
            if half == 1:
                nc.vector.tensor_copy(dst[:, csl], ps[:])

        def _vproj_unit(p, g, half):
            """2 t-tiles of V in [t, dv] layout (x-tile stationary, fp8)."""
            tiles = W_TILES[p]
            V = QKV[p][2]
            xts = {"h": xh, "l": xl}
            if half == 0:
                ps = psum_pr.tile([128, 4, 128], F32, tag="pr",
                                  name=f"psv{p}{g}")
                PROJ_PS[(p, "v", g)] = ps
            else:
                ps = PROJ_PS[(p, "v", g)]
            for j in (0, 1) if half == 0 else (2, 3):
                tt = g * 4 + j
                tsl = slice(tt * 128, (tt + 1) * 128)
                for ti, (xs, ws) in enumerate(TERMS):
                    x_sb = xts[xs]
                    w_sb = tiles["v" + ws]
                    for a in range(0, 8, 2):
                        nc.tensor.matmul(
                            ps[:, j, :], x_sb[:, a:a + 2, tsl],
                            w_sb[:, a:a + 2, :],
                            start=(ti == 0 and a == 0),
                            stop=(ti == 2 and a == 6),
                            perf_mode=DR)
            if half == 1:
                PROJ_PS.pop((p, "v", g))
                nc.vector.tensor_copy(
                    V[:, g * 4:(g + 1) * 4, :, 0:64],
                    ps[:].rearrange("p j (h d) -> p j h d", h=2))

        def qkv_units(p):
            units = []
            for c in range(4):
                for half in range(2):
                    units.append(
                        lambda p=p, c=c, h=half: _qkproj_unit(p, "q", c, h))
                for half in range(2):
                    units.append(
                        lambda p=p, c=c, h=half: _qkproj_unit(p, "k", c, h))
                for half in range(2):
                    units.append(
                        lambda p=p, c=c, h=half: _vproj_unit(p, c, h))
            return units

        class Filler:
            def __init__(self):
                self._q = []
                self._tick = 0
            def extend(self, units):
                self._q.extend(units)
            def pop(self, every=2):
                self._tick += 1
                if self._q and self._tick % every == 0:
                    self._q.pop(0)()
            def flush(self):
                while self._q:
                    self._q.pop(0)()

        def emit_qk_phase(p, qc, ctxq, filler=None):
            QT, KT, V = QKV[p]
            nki = 4 * (qc + 1)
            qlo = qc * 512
            ses = []
            last_pair = (p == NPAIR - 1) and SCHED_P3
            for ki in range(nki):
                gate = nki // 2 if SCHED_CTXI == 2 else 0
                if ctxq and ki >= gate:
                    ctxq.pop(0)()
                elif filler and last_pair:
                    # pair 3: give the normalize->transpose->hi/lo chain a
                    # head start before popping out-proj units
                    if SCHED_P3 == 3:
                        if ki >= 6:
                            filler.pop(every=1)
                    elif ki >= 5:
                        filler.pop(every=1)
                elif filler:
                    filler.pop(every=1 if p == 0 else SCHED_EVERY)
                ksl = slice(ki * 128, (ki + 1) * 128)
                r = ki - 4 * qc        # >= 0 -> diagonal-region ktile
                ci = max(0, r * 128)
                pss = psum_ss.tile([128, 2, 512], F32, tag="ss",
                                   name=f"pss{p}_{qc}_{ki}")
                nc.tensor.matmul(pss[:, 0, ci:], KT[0:64, ksl],
                                 QT[0:64, qlo + ci:qlo + 512],
                                 tile_position=(0, 0))
                nc.tensor.matmul(pss[:, 1, ci:], KT[64:128, ksl],
                                 QT[64:128, qlo + ci:qlo + 512],
                                 tile_position=(64, 0))
                se = se_p.tile([128, 2, 512], BF16, tag="se",
                               name=f"se{p}_{qc}_{ki}")
                nc.scalar.activation(se[:, :, ci:], pss[:, :, ci:],
                                     AF.Exp, scale=2.0 ** -13)
                if r >= 0:
                    tm = trimask[:].unsqueeze(1)
                    nc.vector.tensor_tensor(
                        out=se[:, :, ci:ci + 128], in0=se[:, :, ci:ci + 128],
                        in1=tm.broadcast_to([128, 2, 128]), op=OP.mult)
                ses.append(se)
            return ses

        def ctx_units(p, qc, ses):
            # ctx accumulation: groups strictly sequential per PSUM bank
            # (start=True marks the whole 2KB bank pending-zero). h halves
            # sit in separate banks of one [128, 2, 512] tile.
            V = QKV[p][2]
            box = {}
            def _alloc():
                box["pctx"] = psum_cx.tile([128, 2, 512], F32, tag="cx",
                                           name=f"pctx{p}_{qc}")
            def _grp(j):
                jsl = slice(j * 128, (j + 1) * 128)
                nk = 4 * qc + j
                for h in range(2):
                    dst = box["pctx"][:, h, j * VW:(j + 1) * VW]
                    for ki in range(nk + 1):
                        nc.tensor.matmul(dst, ses[ki][:, h, jsl],
                                         V[:, ki, h, :],
                                         start=(ki == 0), stop=(ki == nk))
            def _unit(j):
                if j == 0:
                    _alloc()
                _grp(j)
            def _unit_fp(j):
                _unit(j)
                if SCHED_JPOP:
                    filler.pop(every=SCHED_EVERY)
            units = [lambda j=j: _unit_fp(j) for j in range(4)]
            units.append(lambda: emit_normalize(p, qc, box["pctx"]))
            return units

        def emit_normalize(p, qc, pctx):
            pv = pctx[:, :, 0:4 * VW].rearrange("p h (j w) -> p h j w", j=4)
            rec = rec_p.tile([128, 4, 2, 1], F32, tag="rec",
                             name=f"rec{p}_{qc}")
            nc.vector.reciprocal(
                rec[:], pv[:, :, :, 64:65].rearrange("p h j w -> p j h w"))
            cb = cb_p.tile([128, 4, 2, 64], BF16, tag="cb",
                           name=f"cb{p}_{qc}")
            nc.vector.tensor_tensor(
                out=cb[:],
                in0=pv[:, :, :, 0:64].rearrange("p h j w -> p j h w"),
                in1=rec[:].broadcast_to([128, 4, 2, 64]),
                op=OP.mult)
            cts = cb_p.tile([128, 512], BF16, tag="cts",
                            name=f"cts{p}_{qc}")
            for j in range(4):
                nc.sync.dma_start_transpose(
                    out=cts[:, j * 128:(j + 1) * 128],
                    in_=cb[:, j].rearrange("p h d -> p (h d)"))
            qsl = slice(qc * 512, (qc + 1) * 512)
            nc.gpsimd.tensor_copy(ctxTh[:, p, qsl], cts[:])
            nc.gpsimd.tensor_tensor(out=ctxTl[:, p, qsl],
                                    in0=cts[:],
                                    in1=ctxTh[:, p, qsl], op=OP.subtract)

        def _outproj_unit(tt, ec, act_evac=False, alt=False):
            tsl = slice(tt * 128, (tt + 1) * 128)
            esl = slice(ec * 512, (ec + 1) * 512)
            if alt:
                # tail: score psum pool is idle; borrow a bank for ring depth
                psoT = psum_ss.tile([128, 2, 512], F32, tag="ss",
                                    name=f"psoT{tt}_{ec}")
                pso = psoT[:, 0, :]
            else:
                pso = psum_pr.tile([128, 512], F32, tag="pr",
                                   name=f"pso{tt}_{ec}")[:]
            cts = {"h": ctxTh, "l": ctxTl}
            wos = WO
            for ti, (cs, ws) in enumerate(TERMS):
                ct, wo = cts[cs], wos[ws]
                for pp in (0, 2):
                    nc.tensor.matmul(pso, ct[:, pp:pp + 2, tsl],
                                     wo[:, pp:pp + 2, esl],
                                     start=(ti == 0 and pp == 0),
                                     stop=(ti == 2 and pp == 2),
                                     perf_mode=DR)
            st = st_p.tile([128, 512], BF16, tag="st", name=f"st{tt}_{ec}")
            if act_evac:
                nc.scalar.activation(st[:], pso, AF.Copy)
            else:
                nc.vector.tensor_copy(st[:], pso)
            nc.sync.dma_start(out=out_d[tsl, esl], in_=st[:])

        def outproj_units(qc):
            ae = qc >= int(os.environ.get('K_AE', '4'))
            alt_row = qc >= int(os.environ.get('K_ALT', '3'))
            return [lambda tt=tt, ec=ec:
                    _outproj_unit(tt, ec, act_evac=ae or ((tt * 2 + ec) % 2
                                  and alt_row),
                                  alt=alt_row and (tt * 2 + ec) % 2 == 1)
                    for tt in range(4 * qc, 4 * (qc + 1)) for ec in range(2)]

        def emit_last_tail(ses):
            """Last chunk (pair 3, qc 3): per-qtile ctx -> normalize ->
            out-proj pipeline so the post-exp tail overlaps on all engines."""
            p, qc = NPAIR - 1, 3
            V = QKV[p][2]
            pctx = psum_cx.tile([128, 2, 512], F32, tag="cx",
                                name=f"pctx{p}_{qc}")
            def _grp(j):
                jsl = slice(j * 128, (j + 1) * 128)
                nk = 4 * qc + j
                for h in range(2):
                    dst = pctx[:, h, j * VW:(j + 1) * VW]
                    for ki in range(nk + 1):
                        nc.tensor.matmul(dst, ses[ki][:, h, jsl],
                                         V[:, ki, h, :],
                                         start=(ki == 0), stop=(ki == nk))
            def _norm(j):
                tt = 4 * qc + j
                rec = rec_p.tile([128, 2, 1], F32, tag="rec",
                                 name=f"recL{j}")
                nc.vector.reciprocal(
                    rec[:], pctx[:, :, j * VW + 64:(j + 1) * VW])
                cb = cb_p.tile([128, 2, 64], BF16, tag="cbL",
                               name=f"cbL{j}")
                nc.vector.tensor_tensor(
                    out=cb[:],
                    in0=pctx[:, :, j * VW:j * VW + 64],
                    in1=rec[:].broadcast_to([128, 2, 64]),
                    op=OP.mult)
                cts = cb_p.tile([128, 128], BF16, tag="ctsL",
                                name=f"ctsL{j}")
                nc.sync.dma_start_transpose(
                    out=cts[:], in_=cb[:].rearrange("p h d -> p (h d)"))
                tsl = slice(tt * 128, (tt + 1) * 128)
                nc.gpsimd.tensor_copy(ctxTh[:, p, tsl], cts[:])
                nc.gpsimd.tensor_tensor(out=ctxTl[:, p, tsl],
                                        in0=cts[:],
                                        in1=ctxTh[:, p, tsl],
                                        op=OP.subtract)
            # skewed pipeline: grp(j+1) runs on the PE while qtile j's
            # normalize chain flows through DVE/DMA/Pool
            _grp(0); _norm(0)
            _grp(1); _norm(1)
            _outproj_unit(12, 0)
            _grp(2); _norm(2)
            _outproj_unit(12, 1); _outproj_unit(13, 0)
            _grp(3); _norm(3)
            _outproj_unit(13, 1); _outproj_unit(14, 0)
            _outproj_unit(14, 1)
            _outproj_unit(15, 0); _outproj_unit(15, 1)

        # ---------- emission schedule ----------
        if SCHED_WARM:
            wps = psum_pr.tile([128, 512], F32, tag="pr", name="warm")
            for i in range(SCHED_WARM):
                nc.tensor.matmul(wps[:, 0:128], trimask[:], trimask[:],
                                 start=(i == 0), stop=(i == SCHED_WARM - 1))
            wsb = cb_p.tile([128, 128], BF16, tag="warm", name="warmsb")
            nc.vector.tensor_copy(wsb[:], wps[:, 0:128])
        emit_w_dma(0)
        for c in range(4):
            emit_x_dma(c)
        for p in range(1, NPAIR):
            emit_w_dma(p)
        emit_wo_dma()
        for p in range(NPAIR):
            alloc_qkv(p)
        # software-pipelined chunks: emit chunk (p, qc)'s QK/exp stream with
        # the PREVIOUS chunk's ctx groups interleaved one-per-ki, so ACT
        # streams exps continuously while the PE chews older ctx matmuls.
        filler = Filler()
        u0 = qkv_units(0)
        nup = 6 if SCHED_EARLY else len(u0)
        for u in u0[:nup]:
            u()
        filler.extend(u0[nup:])
        ctxq = []
        for p in range(NPAIR):
            last_pair = (p == NPAIR - 1)
            if not last_pair:
                filler.extend(qkv_units(p + 1))
            for qc in [int(ch) for ch in SCHED_ROWS]:
                p3 = last_pair and SCHED_P3
                use_ctxq = ctxq if (SCHED_CTXI or (p3 and SCHED_P3 == 1)) \
                    else []
                ses = emit_qk_phase(p, qc, use_ctxq, filler=filler)
                # (ctxq drained below if not interleaved)
                while ctxq:
                    ctxq.pop(0)()
                if p3 and qc == 3:
                    emit_last_tail(ses)
                    ctxq = []
                else:
                    ctxq = ctx_units(p, qc, ses)
                    if last_pair:
                        ctxq.append(
                            lambda qc=qc: filler.extend(outproj_units(qc)))
            if not last_pair:
                filler.flush()  # next pair's QT/KT must exist before its QK
        while ctxq:
            ctxq.pop(0)()
        filler.flush()

    nc.compile()
    _nc_cache = nc
    return nc


def kernel(x, Wq, Wk, Wv, Wo):
    import ml_dtypes

    from concourse.bass_utils import run_bass_kernel_spmd

    F8 = ml_dtypes.float8_e4m3fn
    nc = _build()
    x = np.asarray(x, dtype=np.float32)
    Wq, Wk, Wv, Wo = (np.asarray(w, dtype=np.float32)
                      for w in (Wq, Wk, Wv, Wo))

    def hilo(a):
        hi = a.astype(F8)
        lo = (a - hi.astype(np.float32)).astype(F8)
        return hi, lo

    def pack_w(Wt):
        # 32*Wt: [sl-feat, 1024] -> hi/lo tiles [pair, 128(d%128), 8, 128]
        t = (WSCALE * Wt.T).reshape(8, 128, NPAIR, 128)  # [a, pd, pair, f]
        t = np.ascontiguousarray(t.transpose(2, 1, 0, 3))
        return hilo(t)

    in_maps = []
    for c in range(NCORES):
        b, hg = c // 2, c % 2
        sl = slice(hg * ESH, (hg + 1) * ESH)
        xth, xtl = hilo(np.ascontiguousarray(x[b].T))
        wqh, wql = pack_w(Wq[sl, :])
        wkh, wkl = pack_w(Wk[sl, :])
        wvh, wvl = pack_w(Wv[sl, :])
        wot = (WSCALE * Wo[:, sl].T).reshape(NPAIR, 128, D)
        woh, wol = hilo(np.ascontiguousarray(wot.transpose(1, 0, 2)))
        in_maps.append({
            "xh": xth, "xl": xtl,
            "wqh": wqh, "wql": wql, "wkh": wkh, "wkl": wkl,
            "wvh": wvh, "wvl": wvl,
            "woh": woh, "wol": wol,
        })
    res = run_bass_kernel_spmd(nc, in_maps, list(range(NCORES)))
    outs = [np.asarray(res.results[c]["out"]).astype(np.float32)
            for c in range(NCORES)]
    return np.stack([(outs[2 * b] + outs[2 * b + 1]) * (1.0 / WSCALE)
                     for b in range(B)])


# revision 4
# speedup vs baseline: 1.0783x; 1.0783x over previous
"""Self-contained MHA kernel for Trainium2, 8 NeuronCores — v3.

Problem: B=4, T=2048, D=1024, H=16 causal MHA, fp32, no bias.
Sharding: core c handles batch b=c//2 and head-group hg=c%2 (8 heads = 4
head-pairs), Megatron-style: Wq/Wk/Wv column-sharded, Wo row-sharded; host
sums the two partial outputs per batch (and divides by the 32x weight
pre-scale).

Design notes:
 - projections and out-projection run as fp8e4m3 hi/lo 3-term DoubleRow
   matmuls (2 k-tiles per instruction at 0.5 cyc/row): x = xh+xl, W' = 32W
   = wh+wl (the 32x pre-scale keeps the lo residuals out of fp8 subnormals;
   the scale is undone via the exp scale, the ones column, and a host-side
   divide). Terms: xh*wh + xl*wh + xh*wl.
 - scores S^T[k,q] in bf16 (2 heads via PE quadrants), exp on ACT with
   scale 2^-13 (= 1/8 softmax scale / 32^2), no max subtraction
 - ctx: se tile [k,q] is the STATIONARY, [V|32] the moving operand ->
   ctx[q, dv+1] accumulated over k-tiles, 65 moving rows per (ktile, qtile,
   head), fused denominator column; groups run strictly sequentially per
   PSUM bank (start=True marks the whole 2KB bank pending-zero)
 - V projected directly into [t, dv] layout (x-tile stationary)
 - ctx normalized on DVE (reciprocal + broadcast multiply), transposed
   [q,dv]->[dv,q] by the DMA xbar, then split to fp8 hi/lo for the
   out-projection
"""

import os

import numpy as np

B, T, D, H = 4, 2048, 1024, 16
SCHED_EVERY = int(os.environ.get("K_EVERY", "2"))
SCHED_EARLY = int(os.environ.get("K_EARLY", "1"))
SCHED_CTXI = int(os.environ.get("K_CTXI", "0"))
SCHED_JPOP = int(os.environ.get("K_JPOP", "1"))
SCHED_P3 = int(os.environ.get("K_P3", "0"))
SCHED_WARM = int(os.environ.get("K_WARM", "40"))
SCHED_ROWS = os.environ.get("K_ROWS", "0123")
DK = 64
NCORES = 8
NPAIR = 4        # head-pairs per core
ESH = 512        # output-feature shard per core (8 heads * 64)
VW = 65          # dv + 1 ones column (denominator)
WSCALE = 32.0    # weight pre-scale (keeps fp8 lo parts out of subnormals)

_nc_cache = None


def _build():
    global _nc_cache
    if _nc_cache is not None:
        return _nc_cache

    from contextlib import ExitStack

    import concourse.bacc as bacc
    import concourse.mybir as mybir
    import concourse.tile as tile
    from concourse.masks import make_upper_triangular

    F32 = mybir.dt.float32
    BF16 = mybir.dt.bfloat16
    FP8 = mybir.dt.float8e4
    AF = mybir.ActivationFunctionType
    OP = mybir.AluOpType
    DR = mybir.MatmulPerfMode.DoubleRow

    nc = bacc.Bacc("TRN2", target_bir_lowering=False, debug=False,
                   num_devices=NCORES)
    xh_d = nc.declare_dram_parameter("xh", [D, T], FP8, isOutput=False)
    xl_d = nc.declare_dram_parameter("xl", [D, T], FP8, isOutput=False)
    w_ds = {}
    for wn in ("qh", "ql", "kh", "kl", "vh", "vl"):
        w_ds[wn] = nc.declare_dram_parameter(
            f"w{wn}", [NPAIR, 128, 8, 128], FP8, isOutput=False)
    woh_d = nc.declare_dram_parameter("woh", [128, NPAIR, D], FP8,
                                      isOutput=False)
    wol_d = nc.declare_dram_parameter("wol", [128, NPAIR, D], FP8,
                                      isOutput=False)
    out_d = nc.declare_dram_parameter("out", [T, D], BF16, isOutput=True)

    with tile.TileContext(nc) as tc, ExitStack() as ctx:
        const_p = ctx.enter_context(tc.tile_pool(name="const", bufs=1))
        xt_p = ctx.enter_context(tc.tile_pool(name="xt", bufs=2))
        w_p = ctx.enter_context(tc.tile_pool(name="w", bufs=24))
        wo_p = ctx.enter_context(tc.tile_pool(name="wo", bufs=2))
        qk_p = ctx.enter_context(tc.tile_pool(name="qk", bufs=6))
        v_p = ctx.enter_context(tc.tile_pool(name="v", bufs=4))
        se_p = ctx.enter_context(tc.tile_pool(name="se", bufs=28))
        cu_p = ctx.enter_context(tc.tile_pool(name="cu", bufs=2))
        cb_p = ctx.enter_context(tc.tile_pool(name="cb", bufs=int(os.environ.get("K_CB", "2"))))
        rec_p = ctx.enter_context(tc.tile_pool(name="rec", bufs=int(os.environ.get("K_REC", "2"))))
        ctxT_p = ctx.enter_context(tc.tile_pool(name="ctxT", bufs=1))
        st_p = ctx.enter_context(tc.tile_pool(name="st", bufs=3))
        psum_pr = ctx.enter_context(tc.tile_pool(name="psum_pr", bufs=2,
                                                 space="PSUM"))
        psum_ss = ctx.enter_context(tc.tile_pool(name="psum_ss", bufs=2,
                                                 space="PSUM"))
        psum_cx = ctx.enter_context(tc.tile_pool(name="psum_cx", bufs=1,
                                                 space="PSUM"))

        # constants
        trimask = const_p.tile([128, 128], BF16)
        make_upper_triangular(nc, trimask[:], val=1.0, diag=True)

        # persistent tensors
        xh = xt_p.tile([128, 8, T], FP8, tag="xt", name="xh")
        xl = xt_p.tile([128, 8, T], FP8, tag="xt", name="xl")
        ctxTh = ctxT_p.tile([128, NPAIR, T], FP8, tag="ctxTh", name="ctxTh")
        ctxTl = ctxT_p.tile([128, NPAIR, T], FP8, tag="ctxTl", name="ctxTl")

        W_TILES = {}   # p -> dict of 6 hi/lo weight tiles
        QKV = {}       # p -> (QT, KT, V)
        WO = {}
        PROJ_PS = {}

        def emit_x_dma(c):
            csl = slice(c * 512, (c + 1) * 512)
            for t_sb, t_d in ((xh, xh_d), (xl, xl_d)):
                nc.sync.dma_start(
                    out=t_sb[:, :, csl],
                    in_=t_d[:, csl].rearrange("(a p) t -> p a t", p=128))

        def emit_w_dma(p, order=("qh", "ql", "kh", "kl", "vh", "vl")):
            tiles = {}
            for wn in order:
                w = w_p.tile([128, 8, 128], FP8, tag="w", name=f"w{wn}{p}")
                nc.sync.dma_start(out=w[:], in_=w_ds[wn][p])
                tiles[wn] = w
            W_TILES[p] = tiles

        def emit_x_dma_half(c, which):
            csl = slice(c * 512, (c + 1) * 512)
            t_sb, t_d = (xh, xh_d) if which == "h" else (xl, xl_d)
            nc.sync.dma_start(
                out=t_sb[:, :, csl],
                in_=t_d[:, csl].rearrange("(a p) t -> p a t", p=128))

        def emit_wo_dma():
            woh = wo_p.tile([128, NPAIR, D], FP8, tag="wo", name="woh")
            wol = wo_p.tile([128, NPAIR, D], FP8, tag="wo", name="wol")
            nc.sync.dma_start(out=woh[:], in_=woh_d[:, :, :])
            nc.sync.dma_start(out=wol[:], in_=wol_d[:, :, :])
            WO["h"] = woh
            WO["l"] = wol

        def alloc_qkv(p):
            QT = qk_p.tile([128, T], BF16, tag="qk", name=f"QT{p}")
            KT = qk_p.tile([128, T], BF16, tag="qk", name=f"KT{p}")
            V = v_p.tile([128, 16, 2, VW], BF16, tag="v", name=f"V{p}")
            nc.gpsimd.memset(V[:, :, :, 64:65], WSCALE)
            QKV[p] = (QT, KT, V)

        # hi/lo 3-term schedule: (x_hi, w_hi), (x_lo, w_hi), (x_hi, w_lo)
        TERMS = (("h", "h"), ("l", "h"), ("h", "l"))

        def _qkproj_unit(p, which, c, half):
            """half of a 512-col chunk of the Q or K projection (fp8 DR)."""
            tiles = W_TILES[p]
            QT, KT, V = QKV[p]
            dst = QT if which == "q" else KT
            csl = slice(c * 512, (c + 1) * 512)
            xts = {"h": xh, "l": xl}
            if half == 0:
                ps = psum_pr.tile([128, 512], F32, tag="pr",
                                  name=f"ps{p}{which}{c}")
                PROJ_PS[(p, which, c)] = ps
                terms = TERMS[:2][:1]  # first term only: 4 DR matmuls
                first = True
            else:
                ps = PROJ_PS.pop((p, which, c))
                terms = TERMS[1:]      # remaining two terms: 8 DR matmuls
                first = False
            for ti, (xs, ws) in enumerate(terms):
                w_sb = tiles[which + ws]
                x_sb = xts[xs]
                last_term = (half == 1 and ti == len(terms) - 1)
                for a in range(0, 8, 2):
                    nc.tensor.matmul(ps[:], w_sb[:, a:a + 2, :],
                                     x_sb[:, a:a + 2, csl],
                                     start=(first and a == 0),
                                     stop=(last_term and a == 6),
                                     perf_mode=DR)
            if half == 1:
                nc.vector.tensor_copy(dst[:, csl], ps[:])

        def _vproj_unit(p, g, half):
            """2 t-tiles of V in [t, dv] layout (x-tile stationary, fp8)."""
            tiles = W_TILES[p]
            V = QKV[p][2]
            xts = {"h": xh, "l": xl}
            if half == 0:
                ps = psum_pr.tile([128, 4, 128], F32, tag="pr",
                                  name=f"psv{p}{g}")
                PROJ_PS[(p, "v", g)] = ps
            else:
                ps = PROJ_PS[(p, "v", g)]
            for j in (0, 1) if half == 0 else (2, 3):
                tt = g * 4 + j
                tsl = slice(tt * 128, (tt + 1) * 128)
                for ti, (xs, ws) in enumerate(TERMS):
                    x_sb = xts[xs]
                    w_sb = tiles["v" + ws]
                    for a in range(0, 8, 2):
                        nc.tensor.matmul(
                            ps[:, j, :], x_sb[:, a:a + 2, tsl],
                            w_sb[:, a:a + 2, :],
                            start=(ti == 0 and a == 0),
                            stop=(ti == 2 and a == 6),
                            perf_mode=DR)
            if half == 1:
                PROJ_PS.pop((p, "v", g))
                nc.vector.tensor_copy(
                    V[:, g * 4:(g + 1) * 4, :, 0:64],
                    ps[:].rearrange("p j (h d) -> p j h d", h=2))

        def qkv_units(p):
            units = []
            for c in range(4):
                for half in range(2):
                    units.append(
                        lambda p=p, c=c, h=half: _qkproj_unit(p, "q", c, h))
                for half in range(2):
                    units.append(
                        lambda p=p, c=c, h=half: _qkproj_unit(p, "k", c, h))
                for half in range(2):
                    units.append(
                        lambda p=p, c=c, h=half: _vproj_unit(p, c, h))
            return units

        class Filler:
            def __init__(self):
                self._q = []
                self._tick = 0
            def extend(self, units):
                self._q.extend(units)
            def pop(self, every=2):
                self._tick += 1
                if self._q and self._tick % every == 0:
                    self._q.pop(0)()
            def flush(self):
                while self._q:
                    self._q.pop(0)()

        def emit_qk_phase(p, qc, ctxq, filler=None):
            QT, KT, V = QKV[p]
            nki = 4 * (qc + 1)
            qlo = qc * 512
            ses = []
            last_pair = (p == NPAIR - 1) and SCHED_P3 in (1, 2, 3)
            for ki in range(nki):
                gate = nki // 2 if SCHED_CTXI == 2 else 0
                if ctxq and ki >= gate:
                    ctxq.pop(0)()
                elif filler and last_pair:
                    # pair 3: give the normalize->transpose->hi/lo chain a
                    # head start before popping out-proj units
                    if SCHED_P3 == 3:
                        if ki >= 6:
                            filler.pop(every=1)
                    elif ki >= 5:
                        filler.pop(every=1)
                elif filler:
                    filler.pop(every=1 if p == 0 else SCHED_EVERY)
                ksl = slice(ki * 128, (ki + 1) * 128)
                r = ki - 4 * qc        # >= 0 -> diagonal-region ktile
                ci = max(0, r * 128)
                pss = psum_ss.tile([128, 2, 512], F32, tag="ss",
                                   name=f"pss{p}_{qc}_{ki}")
                nc.tensor.matmul(pss[:, 0, ci:], KT[0:64, ksl],
                                 QT[0:64, qlo + ci:qlo + 512],
                                 tile_position=(0, 0))
                nc.tensor.matmul(pss[:, 1, ci:], KT[64:128, ksl],
                                 QT[64:128, qlo + ci:qlo + 512],
                                 tile_position=(64, 0))
                se = se_p.tile([128, 2, 512], BF16, tag="se",
                               name=f"se{p}_{qc}_{ki}")
                nc.scalar.activation(se[:, :, ci:], pss[:, :, ci:],
                                     AF.Exp, scale=2.0 ** -13)
                if r >= 0:
                    tm = trimask[:].unsqueeze(1)
                    nc.vector.tensor_tensor(
                        out=se[:, :, ci:ci + 128], in0=se[:, :, ci:ci + 128],
                        in1=tm.broadcast_to([128, 2, 128]), op=OP.mult)
                ses.append(se)
            return ses

        def ctx_units(p, qc, ses, per_qtile=False):
            # ctx accumulation: groups strictly sequential per PSUM bank
            # (start=True marks the whole 2KB bank pending-zero). h halves
            # sit in separate banks of one [128, 2, 512] tile.
            V = QKV[p][2]
            box = {}
            def _alloc():
                box["pctx"] = psum_cx.tile([128, 2, 512], F32, tag="cx",
                                           name=f"pctx{p}_{qc}")
            def _grp(j):
                jsl = slice(j * 128, (j + 1) * 128)
                nk = 4 * qc + j
                for h in range(2):
                    dst = box["pctx"][:, h, j * VW:(j + 1) * VW]
                    for ki in range(nk + 1):
                        nc.tensor.matmul(dst, ses[ki][:, h, jsl],
                                         V[:, ki, h, :],
                                         start=(ki == 0), stop=(ki == nk))
            def _unit(j):
                if j == 0:
                    _alloc()
                _grp(j)
            def _unit_fp(j):
                _unit(j)
                if per_qtile:
                    emit_normalize_j(p, qc, box["pctx"], j)
                if SCHED_JPOP:
                    filler.pop(every=SCHED_EVERY)
            units = [lambda j=j: _unit_fp(j) for j in range(4)]
            if not per_qtile:
                units.append(lambda: emit_normalize(p, qc, box["pctx"]))
            return units

        def emit_normalize_j(p, qc, pctx, j):
            tt = 4 * qc + j
            rec = rec_p.tile([128, 2, 1], F32, tag="rec",
                             name=f"recJ{p}_{qc}_{j}")
            nc.vector.reciprocal(
                rec[:], pctx[:, :, j * VW + 64:(j + 1) * VW])
            cb = cb_p.tile([128, 2, 64], BF16, tag="cbJ",
                           name=f"cbJ{p}_{qc}_{j}")
            nc.vector.tensor_tensor(
                out=cb[:],
                in0=pctx[:, :, j * VW:j * VW + 64],
                in1=rec[:].broadcast_to([128, 2, 64]),
                op=OP.mult)
            cts = cb_p.tile([128, 128], BF16, tag="ctsJ",
                            name=f"ctsJ{p}_{qc}_{j}")
            nc.sync.dma_start_transpose(
                out=cts[:], in_=cb[:].rearrange("p h d -> p (h d)"))
            tsl = slice(tt * 128, (tt + 1) * 128)
            nc.gpsimd.tensor_copy(ctxTh[:, p, tsl], cts[:])
            nc.gpsimd.tensor_tensor(out=ctxTl[:, p, tsl],
                                    in0=cts[:],
                                    in1=ctxTh[:, p, tsl],
                                    op=OP.subtract)

        def emit_normalize(p, qc, pctx):
            pv = pctx[:, :, 0:4 * VW].rearrange("p h (j w) -> p h j w", j=4)
            rec = rec_p.tile([128, 4, 2, 1], F32, tag="rec",
                             name=f"rec{p}_{qc}")
            nc.vector.reciprocal(
                rec[:], pv[:, :, :, 64:65].rearrange("p h j w -> p j h w"))
            cb = cb_p.tile([128, 4, 2, 64], BF16, tag="cb",
                           name=f"cb{p}_{qc}")
            nc.vector.tensor_tensor(
                out=cb[:],
                in0=pv[:, :, :, 0:64].rearrange("p h j w -> p j h w"),
                in1=rec[:].broadcast_to([128, 4, 2, 64]),
                op=OP.mult)
            cts = cb_p.tile([128, 512], BF16, tag="cts",
                            name=f"cts{p}_{qc}")
            for j in range(4):
                nc.sync.dma_start_transpose(
                    out=cts[:, j * 128:(j + 1) * 128],
                    in_=cb[:, j].rearrange("p h d -> p (h d)"))
            for j in range(4):
                qs = slice((qc * 4 + j) * 128, (qc * 4 + j + 1) * 128)
                cs = slice(j * 128, (j + 1) * 128)
                nc.gpsimd.tensor_copy(ctxTh[:, p, qs], cts[:, cs])
                nc.gpsimd.tensor_tensor(out=ctxTl[:, p, qs],
                                        in0=cts[:, cs],
                                        in1=ctxTh[:, p, qs],
                                        op=OP.subtract)

        def _outproj_unit(tt, ec, act_evac=False, alt=False):
            tsl = slice(tt * 128, (tt + 1) * 128)
            esl = slice(ec * 512, (ec + 1) * 512)
            if alt:
                # tail: score psum pool is idle; borrow a bank for ring depth
                psoT = psum_ss.tile([128, 2, 512], F32, tag="ss",
                                    name=f"psoT{tt}_{ec}")
                pso = psoT[:, 0, :]
            else:
                pso = psum_pr.tile([128, 512], F32, tag="pr",
                                   name=f"pso{tt}_{ec}")[:]
            cts = {"h": ctxTh, "l": ctxTl}
            wos = WO
            for ti, (cs, ws) in enumerate(
                    (("h", "h"), ("h", "l"), ("l", "h"))):
                ct, wo = cts[cs], wos[ws]
                for pp in (0, 2):
                    nc.tensor.matmul(pso, ct[:, pp:pp + 2, tsl],
                                     wo[:, pp:pp + 2, esl],
                                     start=(ti == 0 and pp == 0),
                                     stop=(ti == 2 and pp == 2),
                                     perf_mode=DR)
            st = st_p.tile([128, 512], BF16, tag="st", name=f"st{tt}_{ec}")
            if act_evac:
                nc.scalar.activation(st[:], pso, AF.Copy)
            else:
                nc.vector.tensor_copy(st[:], pso)
            nc.sync.dma_start(out=out_d[tsl, esl], in_=st[:])

        def outproj_units(qc):
            ae = qc >= int(os.environ.get('K_AE', '4'))
            alt_row = qc >= int(os.environ.get('K_ALT', '2'))
            return [lambda tt=tt, ec=ec:
                    _outproj_unit(tt, ec, act_evac=ae or ((tt * 2 + ec) % 2
                                  and alt_row),
                                  alt=alt_row and (tt * 2 + ec) % 2 == 1)
                    for tt in range(4 * qc, 4 * (qc + 1)) for ec in range(2)]

        def emit_last_tail(ses):
            """Last chunk (pair 3, qc 3): per-qtile ctx -> normalize ->
            out-proj pipeline so the post-exp tail overlaps on all engines."""
            p, qc = NPAIR - 1, 3
            V = QKV[p][2]
            pctx = psum_cx.tile([128, 2, 512], F32, tag="cx",
                                name=f"pctx{p}_{qc}")
            def _grp(j):
                jsl = slice(j * 128, (j + 1) * 128)
                nk = 4 * qc + j
                for h in range(2):
                    dst = pctx[:, h, j * VW:(j + 1) * VW]
                    for ki in range(nk + 1):
                        nc.tensor.matmul(dst, ses[ki][:, h, jsl],
                                         V[:, ki, h, :],
                                         start=(ki == 0), stop=(ki == nk))
            def _norm(j):
                tt = 4 * qc + j
                rec = rec_p.tile([128, 2, 1], F32, tag="rec",
                                 name=f"recL{j}")
                nc.vector.reciprocal(
                    rec[:], pctx[:, :, j * VW + 64:(j + 1) * VW])
                cb = cb_p.tile([128, 2, 64], BF16, tag="cbL",
                               name=f"cbL{j}")
                nc.vector.tensor_tensor(
                    out=cb[:],
                    in0=pctx[:, :, j * VW:j * VW + 64],
                    in1=rec[:].broadcast_to([128, 2, 64]),
                    op=OP.mult)
                cts = cb_p.tile([128, 128], BF16, tag="ctsL",
                                name=f"ctsL{j}")
                nc.sync.dma_start_transpose(
                    out=cts[:], in_=cb[:].rearrange("p h d -> p (h d)"))
                tsl = slice(tt * 128, (tt + 1) * 128)
                nc.gpsimd.tensor_copy(ctxTh[:, p, tsl], cts[:])
                nc.gpsimd.tensor_tensor(out=ctxTl[:, p, tsl],
                                        in0=cts[:],
                                        in1=ctxTh[:, p, tsl],
                                        op=OP.subtract)
            # skewed pipeline: group j+1 runs on the PE while qtile j's
            # normalize chain flows through DVE/DMA/Pool
            _grp(0); _norm(0)
            _grp(1); _norm(1)
            _grp(2); _norm(2)
            _outproj_unit(12, 0, alt=False)
            _outproj_unit(12, 1, act_evac=True, alt=True)
            _grp(3); _norm(3)
            _outproj_unit(13, 0, alt=False)
            _outproj_unit(13, 1, act_evac=True, alt=True)
            _outproj_unit(14, 0, alt=False)
            _outproj_unit(14, 1, act_evac=True, alt=True)
            _outproj_unit(15, 0, alt=False)
            _outproj_unit(15, 1, act_evac=True, alt=True)

        # ---------- emission schedule ----------
        if SCHED_WARM:
            wps = psum_pr.tile([128, 512], F32, tag="pr", name="warm")
            for i in range(SCHED_WARM):
                nc.tensor.matmul(wps[:, 0:128], trimask[:], trimask[:],
                                 start=(i == 0), stop=(i == SCHED_WARM - 1))
            wsb = cb_p.tile([128, 128], BF16, tag="warm", name="warmsb")
            nc.vector.tensor_copy(wsb[:], wps[:, 0:128])
        # startup order: land the first projection's deps (wqh, xh c0) first
        wt0 = {}
        w0 = w_p.tile([128, 8, 128], FP8, tag="w", name="wqh0")
        nc.sync.dma_start(out=w0[:], in_=w_ds["qh"][0])
        wt0["qh"] = w0
        emit_x_dma_half(0, "h")
        for wn in ("ql", "kh", "kl", "vh", "vl"):
            w = w_p.tile([128, 8, 128], FP8, tag="w", name=f"w{wn}0")
            nc.sync.dma_start(out=w[:], in_=w_ds[wn][0])
            wt0[wn] = w
        W_TILES[0] = wt0
        emit_x_dma_half(0, "l")
        for c in range(1, 4):
            emit_x_dma(c)
        for p in range(1, NPAIR):
            emit_w_dma(p)
        emit_wo_dma()
        for p in range(NPAIR):
            alloc_qkv(p)
        # software-pipelined chunks: emit chunk (p, qc)'s QK/exp stream with
        # the PREVIOUS chunk's ctx groups interleaved one-per-ki, so ACT
        # streams exps continuously while the PE chews older ctx matmuls.
        filler = Filler()
        u0 = qkv_units(0)
        nup = 6 if SCHED_EARLY else len(u0)
        for u in u0[:nup]:
            u()
        filler.extend(u0[nup:])
        ctxq = []
        for p in range(NPAIR):
            last_pair = (p == NPAIR - 1)
            if not last_pair:
                filler.extend(qkv_units(p + 1))
            for qc in [int(ch) for ch in SCHED_ROWS]:
                p3 = last_pair and SCHED_P3
                use_ctxq = ctxq if (SCHED_CTXI or (p3 and SCHED_P3 == 1)) \
                    else []
                ses = emit_qk_phase(p, qc, use_ctxq, filler=filler)
                # (ctxq drained below if not interleaved)
                while ctxq:
                    ctxq.pop(0)()
                if (p3 or SCHED_P3 == 4) and qc == 3 and p == NPAIR - 1:
                    emit_last_tail(ses)
                    ctxq = []
                else:
                    pq = int(os.environ.get("K_PQ", "0"))
                    ctxq = ctx_units(p, qc, ses,
                                     per_qtile=(pq == 2) or
                                     (pq == 1 and p == NPAIR - 1 and qc == 3))
                    if last_pair:
                        ctxq.append(
                            lambda qc=qc: filler.extend(outproj_units(qc)))
            if not last_pair:
                filler.flush()  # next pair's QT/KT must exist before its QK
        while ctxq:
            ctxq.pop(0)()
        filler.flush()

    nc.compile()
    _nc_cache = nc
    return nc


def kernel(x, Wq, Wk, Wv, Wo):
    import ml_dtypes

    from concourse.bass_utils import run_bass_kernel_spmd

    F8 = ml_dtypes.float8_e4m3fn
    nc = _build()
    x = np.asarray(x, dtype=np.float32)
    Wq, Wk, Wv, Wo = (np.asarray(w, dtype=np.float32)
                      for w in (Wq, Wk, Wv, Wo))

    def hilo(a):
        hi = a.astype(F8)
        lo = (a - hi.astype(np.float32)).astype(F8)
        return hi, lo

    def pack_w(Wt):
        # 32*Wt: [sl-feat, 1024] -> hi/lo tiles [pair, 128(d%128), 8, 128]
        t = (WSCALE * Wt.T).reshape(8, 128, NPAIR, 128)  # [a, pd, pair, f]
        t = np.ascontiguousarray(t.transpose(2, 1, 0, 3))
        return hilo(t)

    in_maps = []
    for c in range(NCORES):
        b, hg = c // 2, c % 2
        sl = slice(hg * ESH, (hg + 1) * ESH)
        xth, xtl = hilo(np.ascontiguousarray(x[b].T))
        wqh, wql = pack_w(Wq[sl, :])
        wkh, wkl = pack_w(Wk[sl, :])
        wvh, wvl = pack_w(Wv[sl, :])
        wot = (WSCALE * Wo[:, sl].T).reshape(NPAIR, 128, D)
        woh, wol = hilo(np.ascontiguousarray(wot.transpose(1, 0, 2)))
        in_maps.append({
            "xh": xth, "xl": xtl,
            "wqh": wqh, "wql": wql, "wkh": wkh, "wkl": wkl,
            "wvh": wvh, "wvl": wvl,
            "woh": woh, "wol": wol,
        })
    res = run_bass_kernel_spmd(nc, in_maps, list(range(NCORES)))
    outs = [np.asarray(res.results[c]["out"]).astype(np.float32)
            for c in range(NCORES)]
    return np.stack([(outs[2 * b] + outs[2 * b + 1]) * (1.0 / WSCALE)
                     for b in range(B)])
